# revision 8
# baseline (speedup 1.0000x reference)
"""ChamferLoss Trainium2 kernel (8 NeuronCores, bass/Tile) — radius-band version.

pred, target: [2, 16384, 3] fp32 -> scalar fp32
  d[b,n,m] = ||pred[b,n] - target[b,m]||
  out = mean(min_m d) + mean(min_n d)

Algorithm: sort both clouds by radius (host, free). After sorting, the
nearest neighbor of a point is (for well-spread data) within ~2000 ranks,
so each 128-pred block only computes d^2 against a static 6144-wide window
of radius-sorted targets (37.5% of the dense work). The host certifies per
input that every fwd/bwd nearest neighbor lies inside its window using the
triangle inequality |r_p - r_t| <= d(p,t) (outside-window targets differ in
radius by more than an in-window upper bound); uncertifiable inputs fall
back to the dense kernel (build_kernel_dense), which is always correct.

Sharding: core c = (batch b=c//4, pred-quarter q=c%4): 4096 sorted preds x a
10240-wide window of sorted targets. Per core:
  - PE: d^2 tiles via one K=128 bf16 matmul (aug rows replicated 4x to keep
    the PE array dense and the HAM clock at 2.4 GHz; see _aug_* below).
  - ScalarE: PSUM fp32 -> SBUF fp16 conversion (scaled by BOOST/NREP).
  - VectorE: fp16 mins (2x mode): per block, bwd TT-min into a windowed
    bacc [128, 10240], fwd fold of the block's 6144-wide strip to 128 wide;
    one batched reduce at the end gives per-block fwd mins.
  - Tail: PE transposes finalized bacc 512-col spans; VectorE reduce_min.
Host: scatter-min bwd partials by target rank, sqrt + means (O(N) work).
"""

import ml_dtypes
import numpy as np

import concourse.bass as bass
import concourse.tile as tile
from concourse import mybir

F32 = mybir.dt.float32
F16 = mybir.dt.float16
BF16 = mybir.dt.bfloat16

B = 2
N = 16384          # preds per batch
M = 16384          # targets per batch
NQ = N // 4        # preds per core
KA = 30            # base augmented contraction depth
NREP = 4           # replication count (30*4 = 120 <= 128)
K = 128            # padded contraction depth
NB = NQ // 128     # pred blocks per core (32)
MM_N = 512         # matmul free dim (one PSUM bank)
N_CORES = 8
BOOST = 64.0       # pre-conversion scale: keeps tiny d^2 out of fp16
                   # subnormals (max d^2 ~ 300 * 64 still << fp16 max)
BIG = 60000.0      # bacc init (scaled d^2 stays far below this)

W = 6144           # target-window width per pred block (12 x 512)
HALF = W // 2      # virtual window half-width
TAW = 3584 + W     # per-core taug width: off_nb ranges 0..3584
CORE_BASE_OFF = -HALF  # core target base = 4096*q - HALF (virtual rank)

G = 2048           # convert granularity (4 PSUM banks)
NG_DENSE = M // G


def _off(nb):
    # static window column offset of pred block nb inside the core's taug
    return ((128 * nb + 64) // 512) * 512


# --------------------------------------------------------------------------
# Workaround: this walrus build accepts at most one sync-wait command per
# instruction. Hoist extra waits onto same-engine NoOps placed just before.
# --------------------------------------------------------------------------

def _split_sync_waits(nc):
    counter = 0
    for block in nc.m.functions[0].blocks:
        insts = block.instructions
        out = []
        changed = False
        for inst in insts:
            si = inst.sync_info
            if si is not None and si.on_wait and len(si.on_wait) > 1:
                waits = list(si.on_wait)
                for w in waits[:-1]:
                    counter += 1
                    out.append(
                        mybir.InstNoOp(
                            name=f"waitnop-{counter}",
                            engine=inst.engine,
                            sync_info=mybir.SyncInfo(on_wait=[w], on_update=[]),
                        )
                    )
                si.on_wait = waits[-1:]
                changed = True
            out.append(inst)
        if changed:
            block.instructions = out


def _patch_bass():
    if getattr(bass.Bass, "_split_waits_patched", False):
        return
    orig = bass.Bass.to_json_bytes

    def to_json_bytes(self, *a, **kw):
        _split_sync_waits(self)
        return orig(self, *a, **kw)

    bass.Bass.to_json_bytes = to_json_bytes
    bass.Bass._split_waits_patched = True


# --------------------------------------------------------------------------
# Band kernel builder
# --------------------------------------------------------------------------

def build_kernel(n_loop: int = 0):
    """Radius-band kernel. n_loop=0: production straight-line kernel.
    n_loop>0: wrap the (idempotent) compute in a For_i loop for timing."""
    _patch_bass()
    nc = bass.Bass()
    paug_d = nc.dram_tensor("paug", [K, NQ], BF16, kind="ExternalInput")
    taug_d = nc.dram_tensor("taug", [K, TAW], BF16, kind="ExternalInput")
    fmin_d = nc.dram_tensor("fmin", [128, NB], F32, kind="ExternalOutput")
    bmin_d = nc.dram_tensor("bmin", [128, TAW // 128], F32, kind="ExternalOutput")

    CVT_SCALE = BOOST / NREP

    with tile.TileContext(nc) as tc:
        with (
            tc.tile_pool(name="singles", bufs=1) as singles,
            tc.tile_pool(name="work", bufs=3) as work,
        ):
            paug = singles.tile([K, NQ], BF16)
            taug = singles.tile([K, TAW], BF16)
            bacc = singles.tile([128, TAW], F16)
            fm128 = singles.tile([128, NB * 128], F16)
            fmin_sb = singles.tile([128, NB], F32)
            bmin_sb = singles.tile([128, TAW // 128], F32)

            nc.sync.dma_start(out=paug[:], in_=paug_d[:])
            for g in range(TAW // 512):
                nc.sync.dma_start(
                    out=taug[:, g * 512:(g + 1) * 512],
                    in_=taug_d[:, g * 512:(g + 1) * 512],
                )

            ident = singles.tile([128, 128], F16)
            nc.gpsimd.memset(ident[:], 0.0)
            nc.gpsimd.affine_select(
                out=ident[:],
                in_=ident[:],
                compare_op=mybir.AluOpType.not_equal,
                fill=1.0,
                base=0,
                pattern=[[-1, 128]],
                channel_multiplier=1,
            )

            with tc.tile_pool(name="psum", bufs=2, space="PSUM") as psum:
                def tail_fold_span(c0, c1):
                    # bacc cols [c0, c1) are final: transpose 128-col chunks
                    # through PE (4 per PSUM tile), reduce to per-target mins
                    for t4 in range(c0 // 512, c1 // 512):
                        tp = psum.tile([128, 512], F16, name=f"tp{t4}",
                                       tag="d2")
                        for u in range(4):
                            t = t4 * 4 + u
                            nc.tensor.transpose(
                                tp[:, u * 128:(u + 1) * 128],
                                bacc[:, t * 128:(t + 1) * 128],
                                ident[:],
                            )
                        nc.vector.tensor_reduce(
                            out=bmin_sb[:, t4 * 4:(t4 + 1) * 4],
                            in_=tp[:].rearrange("p (u f) -> p u f", u=4),
                            axis=mybir.AxisListType.X,
                            op=mybir.AluOpType.min,
                        )

                def main_compute():
                    # bacc re-init each pass (keeps the loop idempotent-safe
                    # even though re-min would also be correct)
                    for nb in range(NB):
                        off = _off(nb)
                        lhsT = paug[:, nb * 128:(nb + 1) * 128]
                        cvt = work.tile([128, W], F16, name=f"cvt{nb}",
                                        tag="cvt")
                        for h in range(W // G):
                            d2 = psum.tile([128, G], F32, name=f"d2_{nb}_{h}",
                                           tag="d2")
                            for j in range(G // MM_N):
                                c = off + h * G + j * MM_N
                                nc.tensor.matmul(
                                    d2[:, j * MM_N:(j + 1) * MM_N],
                                    lhsT,
                                    taug[:, c:c + MM_N],
                                    start=True,
                                    stop=True,
                                )
                            nc.scalar.activation(
                                out=cvt[:, h * G:(h + 1) * G], in_=d2[:],
                                func=mybir.ActivationFunctionType.Copy,
                                scale=CVT_SCALE,
                            )
                        # backward: min into the windowed accumulator.
                        # Spans covered for the first time are written with a
                        # plain copy (no prior value) — no bacc init needed.
                        cov_end = 0 if nb == 0 else _off(nb - 1) + W
                        new_lo = max(off, cov_end)
                        if new_lo > off:
                            for h in range(W // G):
                                lo, hi = off + h * G, off + (h + 1) * G
                                mn_hi = min(hi, new_lo)
                                if mn_hi > lo:
                                    nc.vector.tensor_tensor(
                                        out=bacc[:, lo:mn_hi],
                                        in0=bacc[:, lo:mn_hi],
                                        in1=cvt[:, lo - off:mn_hi - off],
                                        op=mybir.AluOpType.min,
                                    )
                                if hi > max(lo, new_lo):
                                    cp_lo = max(lo, new_lo)
                                    nc.vector.tensor_copy(
                                        bacc[:, cp_lo:hi],
                                        cvt[:, cp_lo - off:hi - off],
                                    )
                        else:
                            for h in range(W // G):
                                nc.vector.tensor_copy(
                                    bacc[:, off + h * G:off + (h + 1) * G],
                                    cvt[:, h * G:(h + 1) * G],
                                )
                        # forward fold: 6144 -> 128 into fm128 column block
                        f2 = work.tile([128, 2048], F16, name=f"f2_{nb}",
                                       tag="f2")
                        nc.vector.tensor_tensor(
                            out=f2[:], in0=cvt[:, 0:2048],
                            in1=cvt[:, 2048:4096], op=mybir.AluOpType.min,
                        )
                        nc.vector.tensor_tensor(
                            out=f2[:], in0=f2[:], in1=cvt[:, 4096:6144],
                            op=mybir.AluOpType.min,
                        )
                        for wdt in (1024, 512, 256):
                            nc.vector.tensor_tensor(
                                out=f2[:, 0:wdt], in0=f2[:, 0:wdt],
                                in1=f2[:, wdt:2 * wdt],
                                op=mybir.AluOpType.min,
                            )
                        nc.vector.tensor_tensor(
                            out=fm128[:, nb * 128:(nb + 1) * 128],
                            in0=f2[:, 0:128], in1=f2[:, 128:256],
                            op=mybir.AluOpType.min,
                        )
                        # tail: spans that no later block covers are final
                        if nb == NB - 1:
                            tail_fold_span(_off(nb), TAW)
                        elif _off(nb + 1) > off:
                            tail_fold_span(off, _off(nb + 1))
                    # batched fwd reduce: [128, 32, 128] -> [128, 32]
                    nc.vector.tensor_reduce(
                        out=fmin_sb[:],
                        in_=fm128[:].rearrange("p (b f) -> p b f", b=NB),
                        axis=mybir.AxisListType.X,
                        op=mybir.AluOpType.min,
                    )

                if n_loop:
                    with tc.For_i(0, n_loop, 1):
                        main_compute()
                else:
                    main_compute()

            nc.sync.dma_start(out=fmin_d[:], in_=fmin_sb[:])
            nc.sync.dma_start(out=bmin_d[:], in_=bmin_sb[:])
    return nc


# --------------------------------------------------------------------------
# Dense fallback kernel (the original full-cdist version; always correct)
# --------------------------------------------------------------------------

def build_kernel_dense(n_loop: int = 0):
    _patch_bass()
    nc = bass.Bass()
    paug_d = nc.dram_tensor("paug", [K, NQ], BF16, kind="ExternalInput")
    taug_d = nc.dram_tensor("taug", [K, M], BF16, kind="ExternalInput")
    fmin_d = nc.dram_tensor("fmin", [128, NB], F32, kind="ExternalOutput")
    bmin_d = nc.dram_tensor("bmin", [128, M // 128], F32, kind="ExternalOutput")

    with tile.TileContext(nc) as tc:
        with (
            tc.tile_pool(name="singles", bufs=1) as singles,
            tc.tile_pool(name="work", bufs=3) as work,
        ):
            paug = singles.tile([K, NQ], BF16)
            taug = singles.tile([K, M], BF16)
            bacc = singles.tile([128, M], F16)
            fmin_sb = singles.tile([128, NB], F32)
            bmin_sb = singles.tile([128, M // 128], F32)

            nc.sync.dma_start(out=paug[:], in_=paug_d[:])
            for g in range(NG_DENSE):
                nc.sync.dma_start(
                    out=taug[:, g * G:(g + 1) * G],
                    in_=taug_d[:, g * G:(g + 1) * G],
                )

            ident = singles.tile([128, 128], F16)
            nc.gpsimd.memset(ident[:], 0.0)
            nc.gpsimd.affine_select(
                out=ident[:],
                in_=ident[:],
                compare_op=mybir.AluOpType.not_equal,
                fill=1.0,
                base=0,
                pattern=[[-1, 128]],
                channel_multiplier=1,
            )

            CVT_SCALE = BOOST / NREP
            GP = 2 * G

            def main_compute():
                for nb in range(NB):
                    lhsT = paug[:, nb * 128:(nb + 1) * 128]
                    facc = work.tile([128, GP], F16, name=f"facc{nb}",
                                     tag="facc")
                    for gp in range(NG_DENSE // 2):
                        if nb == 0:
                            cvt_pair = bacc[:, gp * GP:(gp + 1) * GP]
                        elif gp == 0:
                            cvt_pair = facc[:]
                        else:
                            cvt_t = work.tile([128, GP], F16,
                                              name=f"cvt{nb}_{gp}", tag="cvt")
                            cvt_pair = cvt_t[:]
                        for h in range(2):
                            g = gp * 2 + h
                            d2 = psum.tile([128, G], F32, name=f"d2_{nb}_{g}",
                                           tag="d2")
                            for j in range(G // MM_N):
                                nc.tensor.matmul(
                                    d2[:, j * MM_N:(j + 1) * MM_N],
                                    lhsT,
                                    taug[:, g * G + j * MM_N:
                                         g * G + (j + 1) * MM_N],
                                    start=True,
                                    stop=True,
                                )
                            nc.scalar.activation(
                                out=cvt_pair[:, h * G:(h + 1) * G], in_=d2[:],
                                func=mybir.ActivationFunctionType.Copy,
                                scale=CVT_SCALE,
                            )
                        if nb == 0:
                            if gp == 0:
                                nc.vector.tensor_copy(facc[:], cvt_pair)
                            else:
                                nc.vector.tensor_tensor(
                                    out=facc[:], in0=facc[:], in1=cvt_pair,
                                    op=mybir.AluOpType.min,
                                )
                        else:
                            nc.vector.tensor_tensor(
                                out=bacc[:, gp * GP:(gp + 1) * GP],
                                in0=bacc[:, gp * GP:(gp + 1) * GP],
                                in1=cvt_pair,
                                op=mybir.AluOpType.min,
                            )
                            if gp != 0:
                                nc.vector.tensor_tensor(
                                    out=facc[:], in0=facc[:], in1=cvt_pair,
                                    op=mybir.AluOpType.min,
                                )
                        if nb == NB - 1:
                            tail_fold_span(gp)
                    nc.vector.tensor_tensor(
                        out=facc[:, 0:2048], in0=facc[:, 0:2048],
                        in1=facc[:, 2048:4096], op=mybir.AluOpType.min,
                    )
                    nc.vector.tensor_tensor(
                        out=facc[:, 0:1024], in0=facc[:, 0:1024],
                        in1=facc[:, 1024:2048], op=mybir.AluOpType.min,
                    )
                    nc.vector.tensor_reduce(
                        out=fmin_sb[:, nb:nb + 1], in_=facc[:, 0:1024],
                        axis=mybir.AxisListType.X, op=mybir.AluOpType.min,
                    )

            with tc.tile_pool(name="psum", bufs=2, space="PSUM") as psum:
                def tail_fold_span(gp):
                    for t4 in range(gp * GP // 512, (gp + 1) * GP // 512):
                        tp = psum.tile([128, 512], F16, name=f"tp{t4}",
                                       tag="d2")
                        for u in range(4):
                            t = t4 * 4 + u
                            nc.tensor.transpose(
                                tp[:, u * 128:(u + 1) * 128],
                                bacc[:, t * 128:(t + 1) * 128],
                                ident[:],
                            )
                        nc.vector.tensor_reduce(
                            out=bmin_sb[:, t4 * 4:(t4 + 1) * 4],
                            in_=tp[:].rearrange("p (u f) -> p u f", u=4),
                            axis=mybir.AxisListType.X,
                            op=mybir.AluOpType.min,
                        )

                if n_loop:
                    with tc.For_i(0, n_loop, 1):
                        main_compute()
                else:
                    main_compute()

            nc.sync.dma_start(out=fmin_d[:], in_=fmin_sb[:])
            nc.sync.dma_start(out=bmin_d[:], in_=bmin_sb[:])
    return nc


# --------------------------------------------------------------------------
# Host-side prep: augmented coordinate matrices. Each fp32 value is split
# into three bf16 terms (h + m + l reproduces the fp32 value to ~2^-24), so
# the expanded d^2 = p2 + t2 - 2 p.t keeps ~fp32-level absolute accuracy.
# Cross terms keep the 8 products with magnitude >= 2^-25 (drop l*l);
# 30 rows total, replicated NREP=4 times and zero-padded to K=128.
# --------------------------------------------------------------------------

def _bf16(x):
    return x.astype(ml_dtypes.bfloat16)


def _split3(x):
    h = _bf16(x)
    r1 = x - h.astype(np.float32)
    m = _bf16(r1)
    l = _bf16(r1 - m.astype(np.float32))
    return h, m, l


def _aug_parts(coords):
    c = coords.astype(np.float32).T  # [3, n]
    n2 = c[0] * c[0] + c[1] * c[1] + c[2] * c[2]  # fp32, matches reference
    return _split3(c), _split3(n2)


def _replicate(base):
    out = np.zeros((K, base.shape[1]), dtype=ml_dtypes.bfloat16)
    for r in range(NREP):
        out[r * KA:(r + 1) * KA] = base
    return out


_CROSS = [(0, 0), (0, 1), (0, 2), (1, 0), (1, 1), (1, 2), (2, 0), (2, 1)]


def _aug_pred(coords):
    (ch, cm, cl), (n2h, n2m, n2l) = _aug_parts(coords)
    terms = [ch, cm, cl]
    base = np.zeros((KA, coords.shape[0]), dtype=ml_dtypes.bfloat16)
    for i, (pi, _) in enumerate(_CROSS):
        base[3 * i:3 * i + 3] = _bf16(-2.0 * terms[pi].astype(np.float32))
    base[24] = n2h
    base[25] = n2m
    base[26] = n2l
    base[27:30] = 1.0
    return _replicate(base)


def _aug_target(coords):
    (ch, cm, cl), (n2h, n2m, n2l) = _aug_parts(coords)
    terms = [ch, cm, cl]
    base = np.zeros((KA, coords.shape[0]), dtype=ml_dtypes.bfloat16)
    for i, (_, ti) in enumerate(_CROSS):
        base[3 * i:3 * i + 3] = terms[ti]
    base[24:27] = 1.0
    base[27] = n2h
    base[28] = n2m
    base[29] = n2l
    return _replicate(base)


# --------------------------------------------------------------------------
# Band construction + certification
# --------------------------------------------------------------------------

def _sorted_clouds(pred, target):
    """Per batch: radius-sorted pred/target clouds."""
    out = []
    for b in range(B):
        p = np.asarray(pred[b], dtype=np.float32)
        t = np.asarray(target[b], dtype=np.float32)
        ps = p[np.argsort(np.linalg.norm(p, axis=1), kind="stable")]
        ts = t[np.argsort(np.linalg.norm(t, axis=1), kind="stable")]
        out.append((ps, ts))
    return out


def _cert_dir(qs, ts, half_cov):
    """True iff for every q in qs the nearest ts-point lies within
    +-half_cov ranks. Vectorized radius bound first; exact brute-force
    check only for the few points the quick bound cannot clear."""
    n, m = len(qs), len(ts)
    rq = np.linalg.norm(qs, axis=1)
    rt = np.linalg.norm(ts, axis=1)
    idx = np.arange(n)
    ctr = np.clip((idx * m) // n, 0, m - 1)
    # upper bound: min distance over a dense rank-aligned candidate window
    ub2 = np.full(n, np.inf, dtype=np.float64)
    for k in range(-256, 257, 2):
        j = np.clip(ctr + k, 0, m - 1)
        d2 = ((qs - ts[j]) ** 2).sum(1, dtype=np.float64)
        ub2 = np.minimum(ub2, d2)
    # outside-window targets differ in radius by at least dr_out
    lo = ctr - half_cov
    hi = ctr + half_cov
    dr_lo = np.where(lo > 0, rq - rt[np.clip(lo, 0, m - 1)], np.inf)
    dr_hi = np.where(hi < m - 1, rt[np.clip(hi, 0, m - 1)] - rq, np.inf)
    dr_out = np.minimum(dr_lo, dr_hi)
    unclear = np.nonzero(ub2 >= dr_out * dr_out * 0.98)[0]
    if len(unclear) == 0:
        return True
    if len(unclear) > 4096:
        return False
    # exact check for the unclear points
    t2 = (ts.astype(np.float64) ** 2).sum(1)
    qs_u = qs[unclear].astype(np.float64)
    d2 = (qs_u ** 2).sum(1)[:, None] + t2[None, :] - 2.0 * qs_u @ ts.T.astype(np.float64)
    nn = d2.argmin(1)
    return bool(np.all(np.abs(nn - ctr[unclear]) <= half_cov))


def certify_band(clouds):
    # conservative guaranteed coverage either side of any point
    half_cov = HALF - 768
    for ps, ts in clouds:
        if not _cert_dir(ps, ts, half_cov):
            return False
        if not _cert_dir(ts, ps, half_cov):
            return False
    return True


def make_in_maps(pred, target):
    """Band-kernel inputs. Also returns nothing extra: the target-rank map
    per core is reconstructed in postprocess from the static layout."""
    clouds = _sorted_clouds(pred, target)
    in_maps = []
    for c in range(N_CORES):
        b, q = divmod(c, 4)
        ps, ts = clouds[b]
        base = 4096 * q + CORE_BASE_OFF
        ranks = np.clip(base + np.arange(TAW), 0, M - 1)
        in_maps.append({
            "paug": _aug_pred(ps[q * NQ:(q + 1) * NQ]),
            "taug": _aug_target(ts[ranks]),
        })
    return in_maps


def make_in_maps_dense(pred, target):
    pred = np.asarray(pred, dtype=np.float32)
    target = np.asarray(target, dtype=np.float32)
    in_maps = []
    taugs = [_aug_target(target[b]) for b in range(B)]
    for c in range(N_CORES):
        b, q = divmod(c, 4)
        in_maps.append({
            "paug": _aug_pred(pred[b, q * NQ:(q + 1) * NQ]),
            "taug": taugs[b],
        })
    return in_maps


def postprocess(results):
    total = np.float64(0.0)
    inv = np.float32(1.0 / BOOST)
    for b in range(B):
        fwd = []
        bwd = np.full(M, np.inf, dtype=np.float32)
        for q in range(4):
            r = results[b * 4 + q]
            fwd.append(np.asarray(r["fmin"]).T.reshape(-1))
            bm = np.asarray(r["bmin"]).T.reshape(-1)   # col j = t*128+p order
            base = 4096 * q + CORE_BASE_OFF
            ranks = np.clip(base + np.arange(TAW), 0, M - 1)
            np.minimum.at(bwd, ranks, bm)
        fwd = np.concatenate(fwd) * inv
        bwd = bwd * inv
        f = np.sqrt(np.maximum(fwd, 0.0, dtype=np.float32)).mean(dtype=np.float64)
        g = np.sqrt(np.maximum(bwd, 0.0, dtype=np.float32)).mean(dtype=np.float64)
        total += (f + g) / B
    return np.asarray(total, dtype=np.float32)


def postprocess_dense(results):
    total = np.float64(0.0)
    for b in range(B):
        fwd = []
        bwd = None
        for q in range(4):
            r = results[b * 4 + q]
            fwd.append(np.asarray(r["fmin"]).T.reshape(-1))
            bm = np.asarray(r["bmin"]).T.reshape(-1)
            bwd = bm if bwd is None else np.minimum(bwd, bm)
        fwd = np.concatenate(fwd) * np.float32(1.0 / BOOST)
        bwd = bwd * np.float32(1.0 / BOOST)
        f = np.sqrt(np.maximum(fwd, 0.0, dtype=np.float32)).mean(dtype=np.float64)
        g = np.sqrt(np.maximum(bwd, 0.0, dtype=np.float32)).mean(dtype=np.float64)
        total += (f + g) / B
    return np.asarray(total, dtype=np.float32)


# --------------------------------------------------------------------------
# PJRT runner (jit built once per process)
# --------------------------------------------------------------------------

def make_runner(nc, n_cores=N_CORES):
    import jax
    from jax.sharding import Mesh, PartitionSpec
    from jax.experimental.shard_map import shard_map
    from concourse.bass2jax import (
        _bass_exec_p,
        install_neuronx_cc_hook,
        partition_id_tensor,
    )

    install_neuronx_cc_hook()
    partition_name = (
        nc.partition_id_tensor.name if nc.partition_id_tensor else None
    )

    in_names, out_names, out_avals, zero_outs = [], [], [], []
    for alloc in nc.m.functions[0].allocations:
        if not isinstance(alloc, mybir.MemoryLocationSet):
            continue
        name = alloc.memorylocations[0].name
        if alloc.kind == "ExternalInput":
            if name != partition_name:
                in_names.append(name)
        elif alloc.kind == "ExternalOutput":
            shape = tuple(alloc.tensor_shape)
            dtype = mybir.dt.np(alloc.dtype)
            out_names.append(name)
            out_avals.append(jax.core.ShapedArray(shape, dtype))
            zero_outs.append(np.zeros(shape, dtype))
    n_params = len(in_names)
    all_in_names = list(in_names) + list(out_names)
    if partition_name is not None:
        all_in_names.append(partition_name)

    def _body(*args):
        operands = list(args)
        if partition_name is not None:
            operands.append(partition_id_tensor())
        outs = _bass_exec_p.bind(
            *operands,
            out_avals=tuple(out_avals),
            in_names=tuple(all_in_names),
            out_names=tuple(out_names),
            lowering_input_output_aliases=(),
            sim_require_finite=True,
            sim_require_nnan=True,
            nc=nc,
        )
        return tuple(outs)

    devices = jax.devices()[:n_cores]
    mesh = Mesh(np.asarray(devices), ("core",))
    in_specs = (PartitionSpec("core"),) * (n_params + len(out_names))
    out_specs = (PartitionSpec("core"),) * len(out_names)
    jitted = jax.jit(
        shard_map(_body, mesh=mesh, in_specs=in_specs, out_specs=out_specs,
                  check_rep=False),
        keep_unused=True,
    )

    dev_cache = {}

    def run(in_maps, cache_key=None):
        import jax as _jax
        from jax.sharding import NamedSharding

        if cache_key is not None and cache_key in dev_cache:
            args = dev_cache[cache_key]
        else:
            concat_in = [
                np.concatenate(
                    [np.asarray(in_maps[c][n]) for c in range(n_cores)], axis=0
                )
                for n in in_names
            ]
            concat_zeros = [
                np.zeros((n_cores * z.shape[0], *z.shape[1:]), z.dtype)
                for z in zero_outs
            ]
            args = concat_in + concat_zeros
            if cache_key is not None:
                sh = NamedSharding(mesh, PartitionSpec("core"))
                args = [_jax.device_put(a, sh) for a in args]
                dev_cache[cache_key] = args
        outs = jitted(*args)
        _jax.block_until_ready(outs)
        return [
            {
                name: np.asarray(outs[i]).reshape(
                    n_cores, *out_avals[i].shape
                )[c]
                for i, name in enumerate(out_names)
            }
            for c in range(n_cores)
        ]

    return run


_CACHE = {}


def kernel(pred, target):
    clouds = _sorted_clouds(pred, target)
    if certify_band(clouds):
        if "run" not in _CACHE:
            _CACHE["run"] = make_runner(build_kernel(0))
        results = _CACHE["run"](make_in_maps(pred, target))
        return postprocess(results)
    # uncertifiable input: dense fallback (always exact)
    if "run_dense" not in _CACHE:
        _CACHE["run_dense"] = make_runner(build_kernel_dense(0))
    results = _CACHE["run_dense"](make_in_maps_dense(pred, target))
    return postprocess_dense(results)


# revision 15
# speedup vs baseline: 1.0293x; 1.0293x over previous
"""ChamferLoss Trainium2 kernel (8 NeuronCores, bass/Tile) — radius-band version.

pred, target: [2, 16384, 3] fp32 -> scalar fp32
  d[b,n,m] = ||pred[b,n] - target[b,m]||
  out = mean(min_m d) + mean(min_n d)

Algorithm: sort both clouds by radius (host, free). After sorting, the
nearest neighbor of a point is (for well-spread data) within ~2000 ranks,
so each 128-pred block only computes d^2 against a static 6144-wide window
of radius-sorted targets (37.5% of the dense work). The host certifies per
input that every fwd/bwd nearest neighbor lies inside its window using the
triangle inequality |r_p - r_t| <= d(p,t) (outside-window targets differ in
radius by more than an in-window upper bound); uncertifiable inputs fall
back to the dense kernel (build_kernel_dense), which is always correct.

Sharding: core c = (batch b=c//4, pred-quarter q=c%4): 4096 sorted preds x a
10240-wide window of sorted targets. Per core:
  - PE: d^2 tiles via one K=128 bf16 matmul (aug rows replicated 4x to keep
    the PE array dense and the HAM clock at 2.4 GHz; see _aug_* below).
  - ScalarE: PSUM fp32 -> SBUF fp16 conversion (scaled by BOOST/NREP).
  - VectorE: fp16 mins (2x mode): per block, bwd TT-min into a windowed
    bacc [128, 10240], fwd fold of the block's 6144-wide strip to 128 wide;
    one batched reduce at the end gives per-block fwd mins.
  - Tail: PE transposes finalized bacc 512-col spans; VectorE reduce_min.
Host: scatter-min bwd partials by target rank, sqrt + means (O(N) work).
"""

import ml_dtypes
import numpy as np

import concourse.bass as bass
import concourse.tile as tile
from concourse import mybir

F32 = mybir.dt.float32
F16 = mybir.dt.float16
BF16 = mybir.dt.bfloat16

B = 2
N = 16384          # preds per batch
M = 16384          # targets per batch
NQ = N // 4        # preds per core
KA = 30            # base augmented contraction depth
NREP = 4           # replication count (30*4 = 120 <= 128)
K = 128            # padded contraction depth
NB = NQ // 128     # pred blocks per core (32)
MM_N = 512         # matmul free dim (one PSUM bank)
N_CORES = 8
BOOST = 64.0       # pre-conversion scale: keeps tiny d^2 out of fp16
                   # subnormals (max d^2 ~ 300 * 64 still << fp16 max)
BIG = 60000.0      # bacc init (scaled d^2 stays far below this)

G = 2048           # convert granularity (4 PSUM banks)
NG_DENSE = M // G

# Window width is selected per input from _W_LADDER (smallest certifiable);
# certification guarantees exactness, so narrower = faster, wider = safer.
_W_LADDER = (5120, 5632, 6144)
_WSEL = {"W": 6144}


def _W():
    return _WSEL["W"]


def _taw(Wv=None):
    return 3584 + (Wv or _W())


def _off(nb):
    # static window column offset of pred block nb inside the core's taug
    return ((128 * nb + 64) // 512) * 512


# --------------------------------------------------------------------------
# Workaround: this walrus build accepts at most one sync-wait command per
# instruction. Hoist extra waits onto same-engine NoOps placed just before.
# --------------------------------------------------------------------------

def _split_sync_waits(nc):
    counter = 0
    for block in nc.m.functions[0].blocks:
        insts = block.instructions
        out = []
        changed = False
        for inst in insts:
            si = inst.sync_info
            if si is not None and si.on_wait and len(si.on_wait) > 1:
                waits = list(si.on_wait)
                for w in waits[:-1]:
                    counter += 1
                    out.append(
                        mybir.InstNoOp(
                            name=f"waitnop-{counter}",
                            engine=inst.engine,
                            sync_info=mybir.SyncInfo(on_wait=[w], on_update=[]),
                        )
                    )
                si.on_wait = waits[-1:]
                changed = True
            out.append(inst)
        if changed:
            block.instructions = out


def _patch_bass():
    if getattr(bass.Bass, "_split_waits_patched", False):
        return
    orig = bass.Bass.to_json_bytes

    def to_json_bytes(self, *a, **kw):
        _split_sync_waits(self)
        return orig(self, *a, **kw)

    bass.Bass.to_json_bytes = to_json_bytes
    bass.Bass._split_waits_patched = True


# --------------------------------------------------------------------------
# Band kernel builder
# --------------------------------------------------------------------------

def build_kernel(n_loop: int = 0, Wv: int | None = None):
    """Radius-band kernel. n_loop=0: production straight-line kernel.
    n_loop>0: wrap the (idempotent) compute in a For_i loop for timing."""
    _patch_bass()
    W = Wv or _W()
    TAW = _taw(W)
    nc = bass.Bass()
    paug_d = nc.dram_tensor("paug", [K, NQ], BF16, kind="ExternalInput")
    taug_d = nc.dram_tensor("taug", [K, TAW], BF16, kind="ExternalInput")
    fmin_d = nc.dram_tensor("fmin", [128, NB], F32, kind="ExternalOutput")
    bmin_d = nc.dram_tensor("bmin", [128, TAW // 128], F32, kind="ExternalOutput")

    CVT_SCALE = BOOST / NREP
    # convert chunks: 2048s plus one 512-multiple remainder
    chunks = [G] * (W // G)
    if W % G:
        chunks.append(W % G)
    # forward fold: halve while > 256; final batched reduce handles the rest
    ffin = W // 2
    while ffin > 256:
        ffin //= 2

    with tile.TileContext(nc) as tc:
        with (
            tc.tile_pool(name="singles", bufs=1) as singles,
            tc.tile_pool(name="work", bufs=3) as work,
        ):
            paug = singles.tile([K, NQ], BF16)
            taug = singles.tile([K, TAW], BF16)
            bacc = singles.tile([128, TAW], F16)
            fmf = singles.tile([128, NB * ffin], F16)
            fmin_sb = singles.tile([128, NB], F32)
            bmin_sb = singles.tile([128, TAW // 128], F32)

            nc.sync.dma_start(out=paug[:], in_=paug_d[:])
            for g in range(TAW // 512):
                nc.sync.dma_start(
                    out=taug[:, g * 512:(g + 1) * 512],
                    in_=taug_d[:, g * 512:(g + 1) * 512],
                )

            ident = singles.tile([128, 128], F16)
            nc.gpsimd.memset(ident[:], 0.0)
            nc.gpsimd.affine_select(
                out=ident[:],
                in_=ident[:],
                compare_op=mybir.AluOpType.not_equal,
                fill=1.0,
                base=0,
                pattern=[[-1, 128]],
                channel_multiplier=1,
            )

            with tc.tile_pool(name="psum", bufs=2, space="PSUM") as psum:
                def tail_fold_span(c0, c1):
                    # bacc cols [c0, c1) are final: transpose 128-col chunks
                    # through PE (4 per PSUM tile), reduce to per-target mins
                    for t4 in range(c0 // 512, c1 // 512):
                        tp = psum.tile([128, 512], F16, name=f"tp{t4}",
                                       tag="d2")
                        for u in range(4):
                            t = t4 * 4 + u
                            nc.tensor.transpose(
                                tp[:, u * 128:(u + 1) * 128],
                                bacc[:, t * 128:(t + 1) * 128],
                                ident[:],
                            )
                        nc.vector.tensor_reduce(
                            out=bmin_sb[:, t4 * 4:(t4 + 1) * 4],
                            in_=tp[:].rearrange("p (u f) -> p u f", u=4),
                            axis=mybir.AxisListType.X,
                            op=mybir.AluOpType.min,
                        )

                def main_compute():
                    for nb in range(NB):
                        off = _off(nb)
                        lhsT = paug[:, nb * 128:(nb + 1) * 128]
                        cvt = work.tile([128, W], F16, name=f"cvt{nb}",
                                        tag="cvt")
                        c0 = 0
                        for h, cw in enumerate(chunks):
                            d2 = psum.tile([128, G], F32, name=f"d2_{nb}_{h}",
                                           tag="d2")
                            for j in range(cw // MM_N):
                                c = off + c0 + j * MM_N
                                nc.tensor.matmul(
                                    d2[:, j * MM_N:(j + 1) * MM_N],
                                    lhsT,
                                    taug[:, c:c + MM_N],
                                    start=True,
                                    stop=True,
                                )
                            nc.scalar.activation(
                                out=cvt[:, c0:c0 + cw], in_=d2[:, 0:cw],
                                func=mybir.ActivationFunctionType.Copy,
                                scale=CVT_SCALE,
                            )
                            c0 += cw
                        # backward: min into the windowed accumulator; spans
                        # covered for the first time get a plain copy (so the
                        # accumulator needs no init)
                        if nb == 0:
                            nc.vector.tensor_copy(bacc[:, off:off + W],
                                                  cvt[:])
                        else:
                            mn_end = min(_off(nb - 1) + W, off + W)
                            nc.vector.tensor_tensor(
                                out=bacc[:, off:mn_end],
                                in0=bacc[:, off:mn_end],
                                in1=cvt[:, 0:mn_end - off],
                                op=mybir.AluOpType.min,
                            )
                            if mn_end < off + W:
                                nc.vector.tensor_copy(
                                    bacc[:, mn_end:off + W],
                                    cvt[:, mn_end - off:W],
                                )
                        # forward fold: W -> ffin into fmf column block
                        f2 = work.tile([128, W // 2], F16, name=f"f2_{nb}",
                                       tag="f2")
                        fw = W // 2
                        nc.vector.tensor_tensor(
                            out=f2[:], in0=cvt[:, 0:fw],
                            in1=cvt[:, fw:W], op=mybir.AluOpType.min,
                        )
                        while fw > 2 * ffin:
                            fw //= 2
                            nc.vector.tensor_tensor(
                                out=f2[:, 0:fw], in0=f2[:, 0:fw],
                                in1=f2[:, fw:2 * fw],
                                op=mybir.AluOpType.min,
                            )
                        nc.vector.tensor_tensor(
                            out=fmf[:, nb * ffin:(nb + 1) * ffin],
                            in0=f2[:, 0:ffin], in1=f2[:, ffin:2 * ffin],
                            op=mybir.AluOpType.min,
                        )
                        # tail: spans that no later block covers are final
                        if nb == NB - 1:
                            tail_fold_span(_off(nb), TAW)
                        elif _off(nb + 1) > off:
                            tail_fold_span(off, _off(nb + 1))
                    # batched fwd reduce: [128, NB, ffin] -> [128, NB]
                    nc.vector.tensor_reduce(
                        out=fmin_sb[:],
                        in_=fmf[:].rearrange("p (b f) -> p b f", b=NB),
                        axis=mybir.AxisListType.X,
                        op=mybir.AluOpType.min,
                    )

                if n_loop:
                    with tc.For_i(0, n_loop, 1):
                        main_compute()
                else:
                    main_compute()

            nc.sync.dma_start(out=fmin_d[:], in_=fmin_sb[:])
            nc.sync.dma_start(out=bmin_d[:], in_=bmin_sb[:])
    return nc


# --------------------------------------------------------------------------
# Dense fallback kernel (the original full-cdist version; always correct)
# --------------------------------------------------------------------------

def build_kernel_dense(n_loop: int = 0):
    _patch_bass()
    nc = bass.Bass()
    paug_d = nc.dram_tensor("paug", [K, NQ], BF16, kind="ExternalInput")
    taug_d = nc.dram_tensor("taug", [K, M], BF16, kind="ExternalInput")
    fmin_d = nc.dram_tensor("fmin", [128, NB], F32, kind="ExternalOutput")
    bmin_d = nc.dram_tensor("bmin", [128, M // 128], F32, kind="ExternalOutput")

    with tile.TileContext(nc) as tc:
        with (
            tc.tile_pool(name="singles", bufs=1) as singles,
            tc.tile_pool(name="work", bufs=3) as work,
        ):
            paug = singles.tile([K, NQ], BF16)
            taug = singles.tile([K, M], BF16)
            bacc = singles.tile([128, M], F16)
            fmin_sb = singles.tile([128, NB], F32)
            bmin_sb = singles.tile([128, M // 128], F32)

            nc.sync.dma_start(out=paug[:], in_=paug_d[:])
            for g in range(NG_DENSE):
                nc.sync.dma_start(
                    out=taug[:, g * G:(g + 1) * G],
                    in_=taug_d[:, g * G:(g + 1) * G],
                )

            ident = singles.tile([128, 128], F16)
            nc.gpsimd.memset(ident[:], 0.0)
            nc.gpsimd.affine_select(
                out=ident[:],
                in_=ident[:],
                compare_op=mybir.AluOpType.not_equal,
                fill=1.0,
                base=0,
                pattern=[[-1, 128]],
                channel_multiplier=1,
            )

            CVT_SCALE = BOOST / NREP
            GP = 2 * G

            def main_compute():
                for nb in range(NB):
                    lhsT = paug[:, nb * 128:(nb + 1) * 128]
                    facc = work.tile([128, GP], F16, name=f"facc{nb}",
                                     tag="facc")
                    for gp in range(NG_DENSE // 2):
                        if nb == 0:
                            cvt_pair = bacc[:, gp * GP:(gp + 1) * GP]
                        elif gp == 0:
                            cvt_pair = facc[:]
                        else:
                            cvt_t = work.tile([128, GP], F16,
                                              name=f"cvt{nb}_{gp}", tag="cvt")
                            cvt_pair = cvt_t[:]
                        for h in range(2):
                            g = gp * 2 + h
                            d2 = psum.tile([128, G], F32, name=f"d2_{nb}_{g}",
                                           tag="d2")
                            for j in range(G // MM_N):
                                nc.tensor.matmul(
                                    d2[:, j * MM_N:(j + 1) * MM_N],
                                    lhsT,
                                    taug[:, g * G + j * MM_N:
                                         g * G + (j + 1) * MM_N],
                                    start=True,
                                    stop=True,
                                )
                            nc.scalar.activation(
                                out=cvt_pair[:, h * G:(h + 1) * G], in_=d2[:],
                                func=mybir.ActivationFunctionType.Copy,
                                scale=CVT_SCALE,
                            )
                        if nb == 0:
                            if gp == 0:
                                nc.vector.tensor_copy(facc[:], cvt_pair)
                            else:
                                nc.vector.tensor_tensor(
                                    out=facc[:], in0=facc[:], in1=cvt_pair,
                                    op=mybir.AluOpType.min,
                                )
                        else:
                            nc.vector.tensor_tensor(
                                out=bacc[:, gp * GP:(gp + 1) * GP],
                                in0=bacc[:, gp * GP:(gp + 1) * GP],
                                in1=cvt_pair,
                                op=mybir.AluOpType.min,
                            )
                            if gp != 0:
                                nc.vector.tensor_tensor(
                                    out=facc[:], in0=facc[:], in1=cvt_pair,
                                    op=mybir.AluOpType.min,
                                )
                        if nb == NB - 1:
                            tail_fold_span(gp)
                    nc.vector.tensor_tensor(
                        out=facc[:, 0:2048], in0=facc[:, 0:2048],
                        in1=facc[:, 2048:4096], op=mybir.AluOpType.min,
                    )
                    nc.vector.tensor_tensor(
                        out=facc[:, 0:1024], in0=facc[:, 0:1024],
                        in1=facc[:, 1024:2048], op=mybir.AluOpType.min,
                    )
                    nc.vector.tensor_reduce(
                        out=fmin_sb[:, nb:nb + 1], in_=facc[:, 0:1024],
                        axis=mybir.AxisListType.X, op=mybir.AluOpType.min,
                    )

            with tc.tile_pool(name="psum", bufs=2, space="PSUM") as psum:
                def tail_fold_span(gp):
                    for t4 in range(gp * GP // 512, (gp + 1) * GP // 512):
                        tp = psum.tile([128, 512], F16, name=f"tp{t4}",
                                       tag="d2")
                        for u in range(4):
                            t = t4 * 4 + u
                            nc.tensor.transpose(
                                tp[:, u * 128:(u + 1) * 128],
                                bacc[:, t * 128:(t + 1) * 128],
                                ident[:],
                            )
                        nc.vector.tensor_reduce(
                            out=bmin_sb[:, t4 * 4:(t4 + 1) * 4],
                            in_=tp[:].rearrange("p (u f) -> p u f", u=4),
                            axis=mybir.AxisListType.X,
                            op=mybir.AluOpType.min,
                        )

                if n_loop:
                    with tc.For_i(0, n_loop, 1):
                        main_compute()
                else:
                    main_compute()

            nc.sync.dma_start(out=fmin_d[:], in_=fmin_sb[:])
            nc.sync.dma_start(out=bmin_d[:], in_=bmin_sb[:])
    return nc


# --------------------------------------------------------------------------
# Host-side prep: augmented coordinate matrices. Each fp32 value is split
# into three bf16 terms (h + m + l reproduces the fp32 value to ~2^-24), so
# the expanded d^2 = p2 + t2 - 2 p.t keeps ~fp32-level absolute accuracy.
# Cross terms keep the 8 products with magnitude >= 2^-25 (drop l*l);
# 30 rows total, replicated NREP=4 times and zero-padded to K=128.
# --------------------------------------------------------------------------

def _bf16(x):
    return x.astype(ml_dtypes.bfloat16)


def _split3(x):
    h = _bf16(x)
    r1 = x - h.astype(np.float32)
    m = _bf16(r1)
    l = _bf16(r1 - m.astype(np.float32))
    return h, m, l


def _aug_parts(coords):
    c = coords.astype(np.float32).T  # [3, n]
    n2 = c[0] * c[0] + c[1] * c[1] + c[2] * c[2]  # fp32, matches reference
    return _split3(c), _split3(n2)


def _replicate(base):
    out = np.zeros((K, base.shape[1]), dtype=ml_dtypes.bfloat16)
    for r in range(NREP):
        out[r * KA:(r + 1) * KA] = base
    return out


_CROSS = [(0, 0), (0, 1), (0, 2), (1, 0), (1, 1), (1, 2), (2, 0), (2, 1)]


def _aug_pred(coords):
    (ch, cm, cl), (n2h, n2m, n2l) = _aug_parts(coords)
    terms = [ch, cm, cl]
    base = np.zeros((KA, coords.shape[0]), dtype=ml_dtypes.bfloat16)
    for i, (pi, _) in enumerate(_CROSS):
        base[3 * i:3 * i + 3] = _bf16(-2.0 * terms[pi].astype(np.float32))
    base[24] = n2h
    base[25] = n2m
    base[26] = n2l
    base[27:30] = 1.0
    return _replicate(base)


def _aug_target(coords):
    (ch, cm, cl), (n2h, n2m, n2l) = _aug_parts(coords)
    terms = [ch, cm, cl]
    base = np.zeros((KA, coords.shape[0]), dtype=ml_dtypes.bfloat16)
    for i, (_, ti) in enumerate(_CROSS):
        base[3 * i:3 * i + 3] = terms[ti]
    base[24:27] = 1.0
    base[27] = n2h
    base[28] = n2m
    base[29] = n2l
    return _replicate(base)


# --------------------------------------------------------------------------
# Band construction + certification
# --------------------------------------------------------------------------

def _sorted_clouds(pred, target):
    """Per batch: radius-sorted pred/target clouds."""
    out = []
    for b in range(B):
        p = np.asarray(pred[b], dtype=np.float32)
        t = np.asarray(target[b], dtype=np.float32)
        ps = p[np.argsort(np.linalg.norm(p, axis=1), kind="stable")]
        ts = t[np.argsort(np.linalg.norm(t, axis=1), kind="stable")]
        out.append((ps, ts))
    return out


class _CertDir:
    """NN rank-offset certification for one (query, store) direction.
    Precomputes a dense rank-aligned candidate upper bound once; check()
    then clears points via the radius triangle inequality and brute-forces
    only the residual unclear points."""

    def __init__(self, qs, ts):
        self.qs, self.ts = qs, ts
        n, m = len(qs), len(ts)
        self.n, self.m = n, m
        self.rq = np.linalg.norm(qs, axis=1)
        self.rt = np.linalg.norm(ts, axis=1)
        self.ctr = np.clip((np.arange(n) * m) // n, 0, m - 1)
        ub2 = np.full(n, np.inf, dtype=np.float64)
        for k in range(-256, 257, 2):
            j = np.clip(self.ctr + k, 0, m - 1)
            d2 = ((qs - ts[j]) ** 2).sum(1, dtype=np.float64)
            ub2 = np.minimum(ub2, d2)
        self.ub2 = ub2
        self._nn_cache = {}

    def check(self, half_cov):
        m = self.m
        lo = self.ctr - half_cov
        hi = self.ctr + half_cov
        dr_lo = np.where(lo > 0, self.rq - self.rt[np.clip(lo, 0, m - 1)],
                         np.inf)
        dr_hi = np.where(hi < m - 1, self.rt[np.clip(hi, 0, m - 1)] - self.rq,
                         np.inf)
        dr_out = np.minimum(dr_lo, dr_hi)
        unclear = np.nonzero(self.ub2 >= dr_out * dr_out * 0.98)[0]
        if len(unclear) == 0:
            return True
        if len(unclear) > 8192:
            return False
        missing = [i for i in unclear if i not in self._nn_cache]
        if missing:
            ts64 = self.ts.astype(np.float64)
            t2 = (ts64 ** 2).sum(1)
            qs_u = self.qs[missing].astype(np.float64)
            d2 = (qs_u ** 2).sum(1)[:, None] + t2[None, :] - 2.0 * qs_u @ ts64.T
            for i, nn in zip(missing, d2.argmin(1)):
                self._nn_cache[i] = nn
        offs = np.array([abs(self._nn_cache[i] - self.ctr[i]) for i in unclear])
        return bool(np.all(offs <= half_cov))


def select_W(clouds):
    """Smallest ladder width whose band provably contains every fwd/bwd
    nearest neighbor; None if even the widest fails (-> dense fallback)."""
    certs = []
    for ps, ts in clouds:
        certs.append(_CertDir(ps, ts))
        certs.append(_CertDir(ts, ps))
    for Wv in _W_LADDER:
        half_cov = Wv // 2 - 640
        if all(c.check(half_cov) for c in certs):
            return Wv
    return None


def make_in_maps(pred, target):
    """Band-kernel inputs (selects and sets the band width _WSEL). The
    target-rank map per core is reconstructed in postprocess from the
    static layout."""
    clouds = _sorted_clouds(pred, target)
    Wv = select_W(clouds)
    if Wv is not None:
        _WSEL["W"] = Wv
    Wv = _W()
    in_maps = []
    for c in range(N_CORES):
        b, q = divmod(c, 4)
        ps, ts = clouds[b]
        base = 4096 * q - Wv // 2
        ranks = np.clip(base + np.arange(_taw(Wv)), 0, M - 1)
        in_maps.append({
            "paug": _aug_pred(ps[q * NQ:(q + 1) * NQ]),
            "taug": _aug_target(ts[ranks]),
        })
    return in_maps


def make_in_maps_dense(pred, target):
    pred = np.asarray(pred, dtype=np.float32)
    target = np.asarray(target, dtype=np.float32)
    in_maps = []
    taugs = [_aug_target(target[b]) for b in range(B)]
    for c in range(N_CORES):
        b, q = divmod(c, 4)
        in_maps.append({
            "paug": _aug_pred(pred[b, q * NQ:(q + 1) * NQ]),
            "taug": taugs[b],
        })
    return in_maps


def postprocess(results):
    total = np.float64(0.0)
    inv = np.float32(1.0 / BOOST)
    Wv = _W()
    for b in range(B):
        fwd = []
        bwd = np.full(M, np.inf, dtype=np.float32)
        for q in range(4):
            r = results[b * 4 + q]
            fwd.append(np.asarray(r["fmin"]).T.reshape(-1))
            bm = np.asarray(r["bmin"]).T.reshape(-1)   # col j = t*128+p order
            base = 4096 * q - Wv // 2
            ranks = np.clip(base + np.arange(_taw(Wv)), 0, M - 1)
            np.minimum.at(bwd, ranks, bm)
        fwd = np.concatenate(fwd) * inv
        bwd = bwd * inv
        f = np.sqrt(np.maximum(fwd, 0.0, dtype=np.float32)).mean(dtype=np.float64)
        g = np.sqrt(np.maximum(bwd, 0.0, dtype=np.float32)).mean(dtype=np.float64)
        total += (f + g) / B
    return np.asarray(total, dtype=np.float32)


def postprocess_dense(results):
    total = np.float64(0.0)
    for b in range(B):
        fwd = []
        bwd = None
        for q in range(4):
            r = results[b * 4 + q]
            fwd.append(np.asarray(r["fmin"]).T.reshape(-1))
            bm = np.asarray(r["bmin"]).T.reshape(-1)
            bwd = bm if bwd is None else np.minimum(bwd, bm)
        fwd = np.concatenate(fwd) * np.float32(1.0 / BOOST)
        bwd = bwd * np.float32(1.0 / BOOST)
        f = np.sqrt(np.maximum(fwd, 0.0, dtype=np.float32)).mean(dtype=np.float64)
        g = np.sqrt(np.maximum(bwd, 0.0, dtype=np.float32)).mean(dtype=np.float64)
        total += (f + g) / B
    return np.asarray(total, dtype=np.float32)


# --------------------------------------------------------------------------
# PJRT runner (jit built once per process)
# --------------------------------------------------------------------------

def make_runner(nc, n_cores=N_CORES):
    import jax
    from jax.sharding import Mesh, PartitionSpec
    from jax.experimental.shard_map import shard_map
    from concourse.bass2jax import (
        _bass_exec_p,
        install_neuronx_cc_hook,
        partition_id_tensor,
    )

    install_neuronx_cc_hook()
    partition_name = (
        nc.partition_id_tensor.name if nc.partition_id_tensor else None
    )

    in_names, out_names, out_avals, zero_outs = [], [], [], []
    for alloc in nc.m.functions[0].allocations:
        if not isinstance(alloc, mybir.MemoryLocationSet):
            continue
        name = alloc.memorylocations[0].name
        if alloc.kind == "ExternalInput":
            if name != partition_name:
                in_names.append(name)
        elif alloc.kind == "ExternalOutput":
            shape = tuple(alloc.tensor_shape)
            dtype = mybir.dt.np(alloc.dtype)
            out_names.append(name)
            out_avals.append(jax.core.ShapedArray(shape, dtype))
            zero_outs.append(np.zeros(shape, dtype))
    n_params = len(in_names)
    all_in_names = list(in_names) + list(out_names)
    if partition_name is not None:
        all_in_names.append(partition_name)

    def _body(*args):
        operands = list(args)
        if partition_name is not None:
            operands.append(partition_id_tensor())
        outs = _bass_exec_p.bind(
            *operands,
            out_avals=tuple(out_avals),
            in_names=tuple(all_in_names),
            out_names=tuple(out_names),
            lowering_input_output_aliases=(),
            sim_require_finite=True,
            sim_require_nnan=True,
            nc=nc,
        )
        return tuple(outs)

    devices = jax.devices()[:n_cores]
    mesh = Mesh(np.asarray(devices), ("core",))
    in_specs = (PartitionSpec("core"),) * (n_params + len(out_names))
    out_specs = (PartitionSpec("core"),) * len(out_names)
    jitted = jax.jit(
        shard_map(_body, mesh=mesh, in_specs=in_specs, out_specs=out_specs,
                  check_rep=False),
        keep_unused=True,
    )

    dev_cache = {}

    def run(in_maps, cache_key=None):
        import jax as _jax
        from jax.sharding import NamedSharding

        if cache_key is not None and cache_key in dev_cache:
            args = dev_cache[cache_key]
        else:
            concat_in = [
                np.concatenate(
                    [np.asarray(in_maps[c][n]) for c in range(n_cores)], axis=0
                )
                for n in in_names
            ]
            concat_zeros = [
                np.zeros((n_cores * z.shape[0], *z.shape[1:]), z.dtype)
                for z in zero_outs
            ]
            args = concat_in + concat_zeros
            if cache_key is not None:
                sh = NamedSharding(mesh, PartitionSpec("core"))
                args = [_jax.device_put(a, sh) for a in args]
                dev_cache[cache_key] = args
        outs = jitted(*args)
        _jax.block_until_ready(outs)
        return [
            {
                name: np.asarray(outs[i]).reshape(
                    n_cores, *out_avals[i].shape
                )[c]
                for i, name in enumerate(out_names)
            }
            for c in range(n_cores)
        ]

    return run


_CACHE = {}


def kernel(pred, target):
    clouds = _sorted_clouds(pred, target)
    Wv = select_W(clouds)
    if Wv is not None:
        _WSEL["W"] = Wv
        key = f"run{Wv}"
        if key not in _CACHE:
            _CACHE[key] = make_runner(build_kernel(0, Wv))
        results = _CACHE[key](make_in_maps(pred, target))
        return postprocess(results)
    # uncertifiable input: dense fallback (always exact)
    if "run_dense" not in _CACHE:
        _CACHE["run_dense"] = make_runner(build_kernel_dense(0))
    results = _CACHE["run_dense"](make_in_maps_dense(pred, target))
    return postprocess_dense(results)


# revision 23
# speedup vs baseline: 1.2154x; 1.1808x over previous
"""ChamferLoss Trainium2 kernel (8 NeuronCores, bass/Tile) — radius-band version.

pred, target: [2, 16384, 3] fp32 -> scalar fp32
  d[b,n,m] = ||pred[b,n] - target[b,m]||
  out = mean(min_m d) + mean(min_n d)

Algorithm: sort both clouds by radius (host, free). After sorting, the
nearest neighbor of a point is (for well-spread data) within ~2000 ranks,
so each 128-pred block only computes d^2 against a static 6144-wide window
of radius-sorted targets (37.5% of the dense work). The host certifies per
input that every fwd/bwd nearest neighbor lies inside its window using the
triangle inequality |r_p - r_t| <= d(p,t) (outside-window targets differ in
radius by more than an in-window upper bound); uncertifiable inputs fall
back to the dense kernel (build_kernel_dense), which is always correct.

Sharding: core c = (batch b=c//4, pred-quarter q=c%4): 4096 sorted preds x a
10240-wide window of sorted targets. Per core:
  - PE: d^2 tiles via one K=128 bf16 matmul (aug rows replicated 4x to keep
    the PE array dense and the HAM clock at 2.4 GHz; see _aug_* below).
  - ScalarE: PSUM fp32 -> SBUF fp16 conversion (scaled by BOOST/NREP).
  - VectorE: fp16 mins (2x mode): per block, bwd TT-min into a windowed
    bacc [128, 10240], fwd fold of the block's 6144-wide strip to 128 wide;
    one batched reduce at the end gives per-block fwd mins.
  - Tail: PE transposes finalized bacc 512-col spans; VectorE reduce_min.
Host: scatter-min bwd partials by target rank, sqrt + means (O(N) work).
"""

import ml_dtypes
import numpy as np

import concourse.bass as bass
import concourse.tile as tile
from concourse import mybir

F32 = mybir.dt.float32
F16 = mybir.dt.float16
BF16 = mybir.dt.bfloat16

B = 2
N = 16384          # preds per batch
M = 16384          # targets per batch
NQ = N // 4        # preds per core
KA = 30            # base augmented contraction depth
NREP = 4           # replication count (30*4 = 120 <= 128)
K = 128            # padded contraction depth
NB = NQ // 128     # pred blocks per core (32)
MM_N = 512         # matmul free dim (one PSUM bank)
N_CORES = 8
BOOST = 64.0       # pre-conversion scale: keeps tiny d^2 out of fp16
                   # subnormals (max d^2 ~ 300 * 64 still << fp16 max)
BIG = 60000.0      # bacc init (scaled d^2 stays far below this)

G = 2048           # convert granularity (4 PSUM banks)
NG_DENSE = M // G

# Window width is selected per input from _W_LADDER (smallest certifiable);
# certification guarantees exactness, so narrower = faster, wider = safer.
_W_LADDER = (5120, 5632, 6144)
_WSEL = {"W": 6144}


def _W():
    return _WSEL["W"]


def _taw(Wv=None):
    return 3584 + (Wv or _W())


def _off(nb):
    # static window column offset of pred block nb inside the core's taug
    return ((128 * nb + 64) // 512) * 512


# --------------------------------------------------------------------------
# Workaround: this walrus build accepts at most one sync-wait command per
# instruction. Hoist extra waits onto same-engine NoOps placed just before.
# --------------------------------------------------------------------------

def _split_sync_waits(nc):
    counter = 0
    for block in nc.m.functions[0].blocks:
        insts = block.instructions
        out = []
        changed = False
        for inst in insts:
            si = inst.sync_info
            if si is not None and si.on_wait and len(si.on_wait) > 1:
                waits = list(si.on_wait)
                for w in waits[:-1]:
                    counter += 1
                    out.append(
                        mybir.InstNoOp(
                            name=f"waitnop-{counter}",
                            engine=inst.engine,
                            sync_info=mybir.SyncInfo(on_wait=[w], on_update=[]),
                        )
                    )
                si.on_wait = waits[-1:]
                changed = True
            out.append(inst)
        if changed:
            block.instructions = out


def _patch_bass():
    if getattr(bass.Bass, "_split_waits_patched", False):
        return
    orig = bass.Bass.to_json_bytes

    def to_json_bytes(self, *a, **kw):
        _split_sync_waits(self)
        return orig(self, *a, **kw)

    bass.Bass.to_json_bytes = to_json_bytes
    bass.Bass._split_waits_patched = True


# --------------------------------------------------------------------------
# Band kernel builder
# --------------------------------------------------------------------------

def build_kernel(n_loop: int = 0, Wv: int | None = None, ablate=()):
    """Radius-band kernel. n_loop=0: production straight-line kernel.
    n_loop>0: wrap the (idempotent) compute in a For_i loop for timing.
    ablate: subset of {'fwd','bwd','tail'} — timing experiments only."""
    _patch_bass()
    W = Wv or _W()
    TAW = _taw(W)
    nc = bass.Bass()
    paug_d = nc.dram_tensor("paug", [K, NQ], BF16, kind="ExternalInput")
    taug_d = nc.dram_tensor("taug", [K, TAW], BF16, kind="ExternalInput")
    fmin_d = nc.dram_tensor("fmin", [128, NB], F32, kind="ExternalOutput")
    bmin_d = nc.dram_tensor("bmin", [128, TAW // 128], F32, kind="ExternalOutput")

    CVT_SCALE = BOOST / NREP
    # convert chunks: 2048s plus one 512-multiple remainder
    chunks = [G] * (W // G)
    if W % G:
        chunks.append(W % G)
    # forward fold: halve while > 256; final batched reduce handles the rest
    ffin = W // 2
    while ffin > 256:
        ffin //= 2

    with tile.TileContext(nc) as tc:
        with (
            tc.tile_pool(name="singles", bufs=1) as singles,
            tc.tile_pool(name="work", bufs=3) as work,
        ):
            paug = singles.tile([K, NQ], BF16)
            taug = singles.tile([K, TAW], BF16)
            bacc = singles.tile([128, TAW], F16)
            if 'fwd' not in ablate:
                fmf = singles.tile([128, NB * ffin], F16)
                fmin_sb = singles.tile([128, NB], F32)
            bmin_sb = singles.tile([128, TAW // 128], F32)

            nc.sync.dma_start(out=paug[:], in_=paug_d[:])
            for g in range(TAW // 512):
                nc.sync.dma_start(
                    out=taug[:, g * 512:(g + 1) * 512],
                    in_=taug_d[:, g * 512:(g + 1) * 512],
                )

            ident = singles.tile([128, 128], F16)
            nc.gpsimd.memset(ident[:], 0.0)
            nc.gpsimd.affine_select(
                out=ident[:],
                in_=ident[:],
                compare_op=mybir.AluOpType.not_equal,
                fill=1.0,
                base=0,
                pattern=[[-1, 128]],
                channel_multiplier=1,
            )

            with tc.tile_pool(name="psum", bufs=2, space="PSUM") as psum:
                def tail_fold_all():
                    # bacc -> per-target mins, run after the main loop when
                    # PSUM and PE are free: transpose 128-col chunks into
                    # [128, 2048] fp16 PSUM tiles (16 chunks each), one
                    # reduce per tile
                    nch = TAW // 128
                    for tb in range((nch + 15) // 16):
                        k = min(16, nch - tb * 16)
                        tp = psum.tile([128, 2048], F16, name=f"tp{tb}",
                                       tag="d2")
                        for u in range(k):
                            t = tb * 16 + u
                            nc.tensor.transpose(
                                tp[:, u * 128:(u + 1) * 128],
                                bacc[:, t * 128:(t + 1) * 128],
                                ident[:],
                            )
                        nc.vector.tensor_reduce(
                            out=bmin_sb[:, tb * 16:tb * 16 + k],
                            in_=tp[:, 0:k * 128].rearrange(
                                "p (u f) -> p u f", u=k),
                            axis=mybir.AxisListType.X,
                            op=mybir.AluOpType.min,
                        )

                def main_compute():
                    for nb in range(NB):
                        off = _off(nb)
                        lhsT = paug[:, nb * 128:(nb + 1) * 128]
                        cvt = work.tile([128, W], F16, name=f"cvt{nb}",
                                        tag="cvt")
                        c0 = 0
                        for h, cw in enumerate(chunks):
                            d2 = psum.tile([128, G],
                                           F16 if 'ps16' in ablate else F32,
                                           name=f"d2_{nb}_{h}", tag="d2")
                            for j in range(cw // MM_N):
                                c = off + c0 + j * MM_N
                                nc.tensor.matmul(
                                    d2[:, j * MM_N:(j + 1) * MM_N],
                                    lhsT,
                                    taug[:, c:c + MM_N],
                                    start=True,
                                    stop=True,
                                )
                            nc.scalar.activation(
                                out=cvt[:, c0:c0 + cw], in_=d2[:, 0:cw],
                                func=mybir.ActivationFunctionType.Copy,
                                scale=CVT_SCALE,
                            )
                            c0 += cw
                        # backward: min into the windowed accumulator; spans
                        # covered for the first time get a plain copy (so the
                        # accumulator needs no init)
                        if 'bwd' in ablate:
                            pass
                        elif nb == 0:
                            nc.vector.tensor_copy(bacc[:, off:off + W],
                                                  cvt[:])
                        else:
                            mn_end = min(_off(nb - 1) + W, off + W)
                            nc.vector.tensor_tensor(
                                out=bacc[:, off:mn_end],
                                in0=bacc[:, off:mn_end],
                                in1=cvt[:, 0:mn_end - off],
                                op=mybir.AluOpType.min,
                            )
                            if mn_end < off + W:
                                nc.vector.tensor_copy(
                                    bacc[:, mn_end:off + W],
                                    cvt[:, mn_end - off:W],
                                )
                        # forward fold: W -> ffin into fmf column block
                        if 'fwd' not in ablate:
                            f2 = work.tile([128, W // 2], F16,
                                           name=f"f2_{nb}", tag="f2")
                            fw = W // 2
                            nc.vector.tensor_tensor(
                                out=f2[:], in0=cvt[:, 0:fw],
                                in1=cvt[:, fw:W], op=mybir.AluOpType.min,
                            )
                            while fw > 2 * ffin:
                                fw //= 2
                                nc.vector.tensor_tensor(
                                    out=f2[:, 0:fw], in0=f2[:, 0:fw],
                                    in1=f2[:, fw:2 * fw],
                                    op=mybir.AluOpType.min,
                                )
                            nc.vector.tensor_tensor(
                                out=fmf[:, nb * ffin:(nb + 1) * ffin],
                                in0=f2[:, 0:ffin], in1=f2[:, ffin:2 * ffin],
                                op=mybir.AluOpType.min,
                            )
                    # tail after the loop: PSUM/PE are free then
                    if 'tail' not in ablate and 'bwd' not in ablate:
                        tail_fold_all()
                    # batched fwd reduce: [128, NB, ffin] -> [128, NB]
                    if 'fwd' in ablate:
                        return
                    nc.vector.tensor_reduce(
                        out=fmin_sb[:],
                        in_=fmf[:].rearrange("p (b f) -> p b f", b=NB),
                        axis=mybir.AxisListType.X,
                        op=mybir.AluOpType.min,
                    )

                if n_loop:
                    with tc.For_i(0, n_loop, 1):
                        main_compute()
                else:
                    main_compute()

            if 'fwd' not in ablate:
                nc.sync.dma_start(out=fmin_d[:], in_=fmin_sb[:])
            if 'bwd' not in ablate and 'tail' not in ablate:
                nc.sync.dma_start(out=bmin_d[:], in_=bmin_sb[:])
    return nc


# --------------------------------------------------------------------------
# Dense fallback kernel (the original full-cdist version; always correct)
# --------------------------------------------------------------------------

def build_kernel_dense(n_loop: int = 0):
    _patch_bass()
    nc = bass.Bass()
    paug_d = nc.dram_tensor("paug", [K, NQ], BF16, kind="ExternalInput")
    taug_d = nc.dram_tensor("taug", [K, M], BF16, kind="ExternalInput")
    fmin_d = nc.dram_tensor("fmin", [128, NB], F32, kind="ExternalOutput")
    bmin_d = nc.dram_tensor("bmin", [128, M // 128], F32, kind="ExternalOutput")

    with tile.TileContext(nc) as tc:
        with (
            tc.tile_pool(name="singles", bufs=1) as singles,
            tc.tile_pool(name="work", bufs=3) as work,
        ):
            paug = singles.tile([K, NQ], BF16)
            taug = singles.tile([K, M], BF16)
            bacc = singles.tile([128, M], F16)
            fmin_sb = singles.tile([128, NB], F32)
            bmin_sb = singles.tile([128, M // 128], F32)

            nc.sync.dma_start(out=paug[:], in_=paug_d[:])
            for g in range(NG_DENSE):
                nc.sync.dma_start(
                    out=taug[:, g * G:(g + 1) * G],
                    in_=taug_d[:, g * G:(g + 1) * G],
                )

            ident = singles.tile([128, 128], F16)
            nc.gpsimd.memset(ident[:], 0.0)
            nc.gpsimd.affine_select(
                out=ident[:],
                in_=ident[:],
                compare_op=mybir.AluOpType.not_equal,
                fill=1.0,
                base=0,
                pattern=[[-1, 128]],
                channel_multiplier=1,
            )

            CVT_SCALE = BOOST / NREP
            GP = 2 * G

            def main_compute():
                for nb in range(NB):
                    lhsT = paug[:, nb * 128:(nb + 1) * 128]
                    facc = work.tile([128, GP], F16, name=f"facc{nb}",
                                     tag="facc")
                    for gp in range(NG_DENSE // 2):
                        if nb == 0:
                            cvt_pair = bacc[:, gp * GP:(gp + 1) * GP]
                        elif gp == 0:
                            cvt_pair = facc[:]
                        else:
                            cvt_t = work.tile([128, GP], F16,
                                              name=f"cvt{nb}_{gp}", tag="cvt")
                            cvt_pair = cvt_t[:]
                        for h in range(2):
                            g = gp * 2 + h
                            d2 = psum.tile([128, G], F32, name=f"d2_{nb}_{g}",
                                           tag="d2")
                            for j in range(G // MM_N):
                                nc.tensor.matmul(
                                    d2[:, j * MM_N:(j + 1) * MM_N],
                                    lhsT,
                                    taug[:, g * G + j * MM_N:
                                         g * G + (j + 1) * MM_N],
                                    start=True,
                                    stop=True,
                                )
                            nc.scalar.activation(
                                out=cvt_pair[:, h * G:(h + 1) * G], in_=d2[:],
                                func=mybir.ActivationFunctionType.Copy,
                                scale=CVT_SCALE,
                            )
                        if nb == 0:
                            if gp == 0:
                                nc.vector.tensor_copy(facc[:], cvt_pair)
                            else:
                                nc.vector.tensor_tensor(
                                    out=facc[:], in0=facc[:], in1=cvt_pair,
                                    op=mybir.AluOpType.min,
                                )
                        else:
                            nc.vector.tensor_tensor(
                                out=bacc[:, gp * GP:(gp + 1) * GP],
                                in0=bacc[:, gp * GP:(gp + 1) * GP],
                                in1=cvt_pair,
                                op=mybir.AluOpType.min,
                            )
                            if gp != 0:
                                nc.vector.tensor_tensor(
                                    out=facc[:], in0=facc[:], in1=cvt_pair,
                                    op=mybir.AluOpType.min,
                                )
                        if nb == NB - 1:
                            tail_fold_span(gp)
                    nc.vector.tensor_tensor(
                        out=facc[:, 0:2048], in0=facc[:, 0:2048],
                        in1=facc[:, 2048:4096], op=mybir.AluOpType.min,
                    )
                    nc.vector.tensor_tensor(
                        out=facc[:, 0:1024], in0=facc[:, 0:1024],
                        in1=facc[:, 1024:2048], op=mybir.AluOpType.min,
                    )
                    nc.vector.tensor_reduce(
                        out=fmin_sb[:, nb:nb + 1], in_=facc[:, 0:1024],
                        axis=mybir.AxisListType.X, op=mybir.AluOpType.min,
                    )

            with tc.tile_pool(name="psum", bufs=2, space="PSUM") as psum:
                def tail_fold_span(gp):
                    for t4 in range(gp * GP // 512, (gp + 1) * GP // 512):
                        tp = psum.tile([128, 512], F16, name=f"tp{t4}",
                                       tag="d2")
                        for u in range(4):
                            t = t4 * 4 + u
                            nc.tensor.transpose(
                                tp[:, u * 128:(u + 1) * 128],
                                bacc[:, t * 128:(t + 1) * 128],
                                ident[:],
                            )
                        nc.vector.tensor_reduce(
                            out=bmin_sb[:, t4 * 4:(t4 + 1) * 4],
                            in_=tp[:].rearrange("p (u f) -> p u f", u=4),
                            axis=mybir.AxisListType.X,
                            op=mybir.AluOpType.min,
                        )

                if n_loop:
                    with tc.For_i(0, n_loop, 1):
                        main_compute()
                else:
                    main_compute()

            nc.sync.dma_start(out=fmin_d[:], in_=fmin_sb[:])
            nc.sync.dma_start(out=bmin_d[:], in_=bmin_sb[:])
    return nc


# --------------------------------------------------------------------------
# Host-side prep: augmented coordinate matrices. Each fp32 value is split
# into three bf16 terms (h + m + l reproduces the fp32 value to ~2^-24), so
# the expanded d^2 = p2 + t2 - 2 p.t keeps ~fp32-level absolute accuracy.
# Cross terms keep the 8 products with magnitude >= 2^-25 (drop l*l);
# 30 rows total, replicated NREP=4 times and zero-padded to K=128.
# --------------------------------------------------------------------------

def _bf16(x):
    return x.astype(ml_dtypes.bfloat16)


def _split3(x):
    h = _bf16(x)
    r1 = x - h.astype(np.float32)
    m = _bf16(r1)
    l = _bf16(r1 - m.astype(np.float32))
    return h, m, l


def _aug_parts(coords):
    c = coords.astype(np.float32).T  # [3, n]
    n2 = c[0] * c[0] + c[1] * c[1] + c[2] * c[2]  # fp32, matches reference
    return _split3(c), _split3(n2)


def _replicate(base):
    out = np.zeros((K, base.shape[1]), dtype=ml_dtypes.bfloat16)
    for r in range(NREP):
        out[r * KA:(r + 1) * KA] = base
    return out


_CROSS = [(0, 0), (0, 1), (0, 2), (1, 0), (1, 1), (1, 2), (2, 0), (2, 1)]


def _aug_pred(coords):
    (ch, cm, cl), (n2h, n2m, n2l) = _aug_parts(coords)
    terms = [ch, cm, cl]
    base = np.zeros((KA, coords.shape[0]), dtype=ml_dtypes.bfloat16)
    for i, (pi, _) in enumerate(_CROSS):
        base[3 * i:3 * i + 3] = _bf16(-2.0 * terms[pi].astype(np.float32))
    base[24] = n2h
    base[25] = n2m
    base[26] = n2l
    base[27:30] = 1.0
    return _replicate(base)


def _aug_target(coords):
    (ch, cm, cl), (n2h, n2m, n2l) = _aug_parts(coords)
    terms = [ch, cm, cl]
    base = np.zeros((KA, coords.shape[0]), dtype=ml_dtypes.bfloat16)
    for i, (_, ti) in enumerate(_CROSS):
        base[3 * i:3 * i + 3] = terms[ti]
    base[24:27] = 1.0
    base[27] = n2h
    base[28] = n2m
    base[29] = n2l
    return _replicate(base)


# --------------------------------------------------------------------------
# Band construction + certification
# --------------------------------------------------------------------------

def _sorted_clouds(pred, target):
    """Per batch: radius-sorted pred/target clouds."""
    out = []
    for b in range(B):
        p = np.asarray(pred[b], dtype=np.float32)
        t = np.asarray(target[b], dtype=np.float32)
        ps = p[np.argsort(np.linalg.norm(p, axis=1), kind="stable")]
        ts = t[np.argsort(np.linalg.norm(t, axis=1), kind="stable")]
        out.append((ps, ts))
    return out


class _CertDir:
    """NN rank-offset certification for one (query, store) direction.
    Precomputes a dense rank-aligned candidate upper bound once; check()
    then clears points via the radius triangle inequality and brute-forces
    only the residual unclear points."""

    def __init__(self, qs, ts):
        self.qs, self.ts = qs, ts
        n, m = len(qs), len(ts)
        self.n, self.m = n, m
        self.rq = np.linalg.norm(qs, axis=1)
        self.rt = np.linalg.norm(ts, axis=1)
        self.ctr = np.clip((np.arange(n) * m) // n, 0, m - 1)
        ub2 = np.full(n, np.inf, dtype=np.float64)
        for k in range(-256, 257, 2):
            j = np.clip(self.ctr + k, 0, m - 1)
            d2 = ((qs - ts[j]) ** 2).sum(1, dtype=np.float64)
            ub2 = np.minimum(ub2, d2)
        self.ub2 = ub2
        self._nn_cache = {}

    def check(self, half_cov):
        m = self.m
        lo = self.ctr - half_cov
        hi = self.ctr + half_cov
        dr_lo = np.where(lo > 0, self.rq - self.rt[np.clip(lo, 0, m - 1)],
                         np.inf)
        dr_hi = np.where(hi < m - 1, self.rt[np.clip(hi, 0, m - 1)] - self.rq,
                         np.inf)
        dr_out = np.minimum(dr_lo, dr_hi)
        unclear = np.nonzero(self.ub2 >= dr_out * dr_out * 0.98)[0]
        if len(unclear) == 0:
            return True
        if len(unclear) > 8192:
            return False
        missing = [i for i in unclear if i not in self._nn_cache]
        if missing:
            ts64 = self.ts.astype(np.float64)
            t2 = (ts64 ** 2).sum(1)
            qs_u = self.qs[missing].astype(np.float64)
            d2 = (qs_u ** 2).sum(1)[:, None] + t2[None, :] - 2.0 * qs_u @ ts64.T
            for i, nn in zip(missing, d2.argmin(1)):
                self._nn_cache[i] = nn
        offs = np.array([abs(self._nn_cache[i] - self.ctr[i]) for i in unclear])
        return bool(np.all(offs <= half_cov))


def select_W(clouds):
    """Smallest ladder width whose band provably contains every fwd/bwd
    nearest neighbor; None if even the widest fails (-> dense fallback)."""
    certs = []
    for ps, ts in clouds:
        certs.append(_CertDir(ps, ts))
        certs.append(_CertDir(ts, ps))
    for Wv in _W_LADDER:
        half_cov = Wv // 2 - 640
        if all(c.check(half_cov) for c in certs):
            return Wv
    return None


def make_in_maps(pred, target):
    """Band-kernel inputs (selects and sets the band width _WSEL). The
    target-rank map per core is reconstructed in postprocess from the
    static layout."""
    clouds = _sorted_clouds(pred, target)
    Wv = select_W(clouds)
    if Wv is not None:
        _WSEL["W"] = Wv
    Wv = _W()
    in_maps = []
    for c in range(N_CORES):
        b, q = divmod(c, 4)
        ps, ts = clouds[b]
        base = 4096 * q - Wv // 2
        ranks = np.clip(base + np.arange(_taw(Wv)), 0, M - 1)
        in_maps.append({
            "paug": _aug_pred(ps[q * NQ:(q + 1) * NQ]),
            "taug": _aug_target(ts[ranks]),
        })
    return in_maps


def make_in_maps_dense(pred, target):
    pred = np.asarray(pred, dtype=np.float32)
    target = np.asarray(target, dtype=np.float32)
    in_maps = []
    taugs = [_aug_target(target[b]) for b in range(B)]
    for c in range(N_CORES):
        b, q = divmod(c, 4)
        in_maps.append({
            "paug": _aug_pred(pred[b, q * NQ:(q + 1) * NQ]),
            "taug": taugs[b],
        })
    return in_maps


def postprocess(results):
    total = np.float64(0.0)
    inv = np.float32(1.0 / BOOST)
    Wv = _W()
    for b in range(B):
        fwd = []
        bwd = np.full(M, np.inf, dtype=np.float32)
        for q in range(4):
            r = results[b * 4 + q]
            fwd.append(np.asarray(r["fmin"]).T.reshape(-1))
            bm = np.asarray(r["bmin"]).T.reshape(-1)   # col j = t*128+p order
            base = 4096 * q - Wv // 2
            ranks = np.clip(base + np.arange(_taw(Wv)), 0, M - 1)
            np.minimum.at(bwd, ranks, bm)
        fwd = np.concatenate(fwd) * inv
        bwd = bwd * inv
        f = np.sqrt(np.maximum(fwd, 0.0, dtype=np.float32)).mean(dtype=np.float64)
        g = np.sqrt(np.maximum(bwd, 0.0, dtype=np.float32)).mean(dtype=np.float64)
        total += (f + g) / B
    return np.asarray(total, dtype=np.float32)


def postprocess_dense(results):
    total = np.float64(0.0)
    for b in range(B):
        fwd = []
        bwd = None
        for q in range(4):
            r = results[b * 4 + q]
            fwd.append(np.asarray(r["fmin"]).T.reshape(-1))
            bm = np.asarray(r["bmin"]).T.reshape(-1)
            bwd = bm if bwd is None else np.minimum(bwd, bm)
        fwd = np.concatenate(fwd) * np.float32(1.0 / BOOST)
        bwd = bwd * np.float32(1.0 / BOOST)
        f = np.sqrt(np.maximum(fwd, 0.0, dtype=np.float32)).mean(dtype=np.float64)
        g = np.sqrt(np.maximum(bwd, 0.0, dtype=np.float32)).mean(dtype=np.float64)
        total += (f + g) / B
    return np.asarray(total, dtype=np.float32)


# --------------------------------------------------------------------------
# PJRT runner (jit built once per process)
# --------------------------------------------------------------------------

def make_runner(nc, n_cores=N_CORES):
    import jax
    from jax.sharding import Mesh, PartitionSpec
    from jax.experimental.shard_map import shard_map
    from concourse.bass2jax import (
        _bass_exec_p,
        install_neuronx_cc_hook,
        partition_id_tensor,
    )

    install_neuronx_cc_hook()
    partition_name = (
        nc.partition_id_tensor.name if nc.partition_id_tensor else None
    )

    in_names, out_names, out_avals, zero_outs = [], [], [], []
    for alloc in nc.m.functions[0].allocations:
        if not isinstance(alloc, mybir.MemoryLocationSet):
            continue
        name = alloc.memorylocations[0].name
        if alloc.kind == "ExternalInput":
            if name != partition_name:
                in_names.append(name)
        elif alloc.kind == "ExternalOutput":
            shape = tuple(alloc.tensor_shape)
            dtype = mybir.dt.np(alloc.dtype)
            out_names.append(name)
            out_avals.append(jax.core.ShapedArray(shape, dtype))
            zero_outs.append(np.zeros(shape, dtype))
    n_params = len(in_names)
    all_in_names = list(in_names) + list(out_names)
    if partition_name is not None:
        all_in_names.append(partition_name)

    def _body(*args):
        operands = list(args)
        if partition_name is not None:
            operands.append(partition_id_tensor())
        outs = _bass_exec_p.bind(
            *operands,
            out_avals=tuple(out_avals),
            in_names=tuple(all_in_names),
            out_names=tuple(out_names),
            lowering_input_output_aliases=(),
            sim_require_finite=True,
            sim_require_nnan=True,
            nc=nc,
        )
        return tuple(outs)

    devices = jax.devices()[:n_cores]
    mesh = Mesh(np.asarray(devices), ("core",))
    in_specs = (PartitionSpec("core"),) * (n_params + len(out_names))
    out_specs = (PartitionSpec("core"),) * len(out_names)
    jitted = jax.jit(
        shard_map(_body, mesh=mesh, in_specs=in_specs, out_specs=out_specs,
                  check_rep=False),
        keep_unused=True,
    )

    dev_cache = {}

    def run(in_maps, cache_key=None):
        import jax as _jax
        from jax.sharding import NamedSharding

        if cache_key is not None and cache_key in dev_cache:
            args = dev_cache[cache_key]
        else:
            concat_in = [
                np.concatenate(
                    [np.asarray(in_maps[c][n]) for c in range(n_cores)], axis=0
                )
                for n in in_names
            ]
            concat_zeros = [
                np.zeros((n_cores * z.shape[0], *z.shape[1:]), z.dtype)
                for z in zero_outs
            ]
            args = concat_in + concat_zeros
            if cache_key is not None:
                sh = NamedSharding(mesh, PartitionSpec("core"))
                args = [_jax.device_put(a, sh) for a in args]
                dev_cache[cache_key] = args
        outs = jitted(*args)
        _jax.block_until_ready(outs)
        return [
            {
                name: np.asarray(outs[i]).reshape(
                    n_cores, *out_avals[i].shape
                )[c]
                for i, name in enumerate(out_names)
            }
            for c in range(n_cores)
        ]

    return run


_CACHE = {}


def kernel(pred, target):
    clouds = _sorted_clouds(pred, target)
    Wv = select_W(clouds)
    if Wv is not None:
        _WSEL["W"] = Wv
        key = f"run{Wv}"
        if key not in _CACHE:
            _CACHE[key] = make_runner(build_kernel(0, Wv))
        results = _CACHE[key](make_in_maps(pred, target))
        return postprocess(results)
    # uncertifiable input: dense fallback (always exact)
    if "run_dense" not in _CACHE:
        _CACHE["run_dense"] = make_runner(build_kernel_dense(0))
    results = _CACHE["run_dense"](make_in_maps_dense(pred, target))
    return postprocess_dense(results)
